# revision 24
# baseline (speedup 1.0000x reference)
"""TRN2 Bass kernel for nn_GQA_22436909154699.

Reference math: softmax over a size-1 axis is identically 1.0, so
    out[b,l,g,h,:] = v[b,l,g,:]          (v = v-half of x @ Wkv + bkv)
The q projection (x @ Wq) never affects the output.  The kernel computes
    res = x @ Wv + bv                    (K=2048, N=256)
data-parallel over tokens across 8 NeuronCores (2048 tokens each), then
broadcasts each group's 64-wide v vector across the 8 heads-per-group
on-chip before a contiguous store.

Precision: matmul runs in bf16 (fp32 PSUM accumulation); the result is
stored bf16 and upcast to fp32 on the host during the gather.  End-to-end
relative error vs the fp32 reference is ~2.9e-3 (harness gate: 2e-2).

Per-core pipeline (TT=16 token tiles of 128 tokens):
  - x is host-cast to bf16 and pre-transposed into [tt, k, ko, t] tiles so
    every DMA lands contiguous per SBUF partition (k mod 128 on partitions).
  - token tiles are processed in groups [1,3,4,4,3,1]: a small first group
    so the first matmuls start after ~1 MB of DMA, a small last group so
    the final store (which serializes after the last compute) is short.
  - all loads stream on the SP HWDGE ring in FIFO order wv[k<8], x-group0,
    wv[k>=8], x-group1..; stores stream on the ACT ring.  (Splitting either
    direction across both rings measured slower - the per-ring rate halves.)
  - ~12 dead 128x512 matmuls on a zeroed tile pre-warm the PE's HAM clock
    gate (cold 1.2 GHz -> warm 2.4 GHz needs ~3.4us of sustained activity)
    while the first loads are in flight, timed to end as group 0 lands.
  - per token tile: 16 bf16 matmuls accumulate v into PSUM; a 1x-rate DVE
    add applies the bias while downcasting PSUM->SBUF bf16 [128,256]; a
    4x-rate bf16 DVE copy broadcasts groups across heads into [128,2048].

Measured on 8 axon trn2 cores: 58.3-64.3 us HW exec (run-to-run window
drift) vs 144.4 us for the staged fp32 baseline.  The body is at the HBM
roofline: ~18 MB/core of unavoidable traffic at ~360-420 GB/s, with the
~28.5 us of PE work and ~19 us of DVE work hidden beneath it; the rest is
the fixed Tile/NRT preamble + drain barriers (~10 us).
"""

import os

import numpy as np
import ml_dtypes

# Problem constants (hardcoded; harness runs kernel.py standalone).
B, L, E = 4, 4096, 2048
G, HPG, D = 4, 8, 64
NV = G * D  # 256 v-columns
NCORES = 8
TOK = B * L  # 16384 tokens
TPC = TOK // NCORES  # 2048 tokens per core
TT = TPC // 128  # 16 token tiles per core
KO = E // 128  # 16 contraction tiles

BF16NP = ml_dtypes.bfloat16

_CACHE: dict = {}
LAST_RESULTS = None


def _build(tpl: int, bcast4d: bool):
    import concourse.bacc as bacc
    import concourse.mybir as mybir
    import concourse.tile as tile

    F32 = mybir.dt.float32
    BF16 = mybir.dt.bfloat16

    nc = bacc.Bacc(
        "TRN2", target_bir_lowering=False, debug=False, num_devices=NCORES
    )
    xt_d = nc.dram_tensor("xt", [TT, 128, KO, 128], BF16, kind="ExternalInput")
    wv_d = nc.dram_tensor("wv", [128, KO, NV], BF16, kind="ExternalInput")
    bias_d = nc.dram_tensor("bias", [128, NV], F32, kind="ExternalInput")
    out_d = nc.dram_tensor("out", [TT, 128, E], BF16, kind="ExternalOutput")

    # Token-tile group sizes: small leading groups so the first matmuls
    # start as soon as ~1 MB has landed; small trailing groups so the final
    # store (which serializes after the last compute) is short.
    groups = {
        1: [1] * TT,
        2: [2] * (TT // 2),
        4: [4] * (TT // 4),
        0: [1, 3, 4, 4, 3, 1],
    }[tpl]
    assert sum(groups) == TT
    maxg = max(groups)
    KH = KO // 2
    nwarm = int(os.environ.get("GQA_WARM", "12"))
    with tile.TileContext(nc) as tc:
        with (
            tc.tile_pool(name="const", bufs=1) as cpool,
            tc.tile_pool(name="xin", bufs=len(groups)) as xpool,
            tc.tile_pool(name="vsb", bufs=4) as vpool,
            tc.tile_pool(name="obuf", bufs=3) as opool,
            tc.tile_pool(name="ps", bufs=4, space="PSUM") as ppool,
            tc.tile_pool(name="warm", bufs=1, space="PSUM") as wpool,
        ):
            # SP-ring order: wv_a, x group 0, wv_b, x group 1.. — the first 8
            # matmuls (k<8) need only wv_a + the 1-tile group 0 (~1 MB); wv_b
            # lands while they run.  bias rides the otherwise-idle ACT ring.
            # (Splitting loads across both rings was measured slower: each
            # ring drops to ~170 GB/s when interleaved with store traffic.)
            wv_a = cpool.tile([128, KH, NV], BF16)
            nc.sync.dma_start(wv_a[:], wv_d[:, :KH])
            wv_b = cpool.tile([128, KH, NV], BF16)
            bias_sb = cpool.tile([128, NV], F32)
            nc.scalar.dma_start(bias_sb[:], bias_d[:])

            # Pre-warm the PE's HAM clock gate (cold 1.2 GHz -> warm 2.4 GHz
            # takes ~3.4us of activity) with dead matmuls on a zeroed tile
            # while the first loads are in flight; sized to end right as the
            # first real matmul's data lands (idle >3.4us re-throttles).
            warm_sb = cpool.tile([128, 512], BF16)
            nc.gpsimd.memset(warm_sb[:], 0.0)
            warm_ps = wpool.tile([128, 512], F32, tag="warm")
            for _ in range(nwarm):
                nc.tensor.matmul(
                    warm_ps[:], lhsT=warm_sb[:, :128], rhs=warm_sb[:],
                    start=True, stop=True,
                )

            t0 = 0
            for gi, gsz in enumerate(groups):
                xin = xpool.tile([128, maxg, KO, 128], BF16, tag="xin")
                nc.sync.dma_start(
                    xin[:, :gsz],
                    xt_d[t0 : t0 + gsz].rearrange("t2 p ko t -> p t2 ko t"),
                )
                if gi == 0:
                    nc.sync.dma_start(wv_b[:], wv_d[:, KH:])
                ot = opool.tile([128, maxg, E], BF16, tag="ot")
                for i in range(gsz):
                    ps = ppool.tile([128, NV], F32, tag="ps")
                    for k in range(KO):
                        rhs = wv_a[:, k, :] if k < KH else wv_b[:, k - KH, :]
                        nc.tensor.matmul(
                            ps[:],
                            lhsT=xin[:, i, k, :],
                            rhs=rhs,
                            start=(k == 0),
                            stop=(k == KO - 1),
                        )
                    vsb = vpool.tile([128, NV], BF16, tag="vsb")
                    nc.vector.tensor_add(vsb[:], ps[:], bias_sb[:])
                    v_g = vsb[:].rearrange("p (g d) -> p g d", g=G)
                    o_g = ot[:, i].rearrange("p (g h d) -> p g h d", g=G, h=HPG)
                    if bcast4d:
                        nc.vector.tensor_copy(
                            o_g,
                            v_g[:, :, None, :].to_broadcast([128, G, HPG, D]),
                        )
                    else:
                        for g in range(G):
                            nc.vector.tensor_copy(
                                o_g[:, g],
                                v_g[:, g, None, :].to_broadcast([128, HPG, D]),
                            )
                nc.scalar.dma_start(
                    out_d[t0 : t0 + gsz].rearrange("t2 p e -> p t2 e"),
                    ot[:, :gsz],
                )
                t0 += gsz
    nc.compile()
    return nc


def _get_nc():
    tpl = int(os.environ.get("GQA_TPL", "0"))
    key = ("nc", tpl)
    if key not in _CACHE:
        try:
            _CACHE[key] = _build(tpl, bcast4d=True)
        except Exception:
            _CACHE[key] = _build(tpl, bcast4d=False)
    return _CACHE[key]


def _prep_inputs(x, Wkv, bkv):
    x = np.asarray(x, dtype=np.float32)
    Wkv = np.asarray(Wkv, dtype=np.float32)
    bkv = np.asarray(bkv, dtype=np.float32)

    # v-columns of the kv projection: Wkv reshaped (E, G, 2, D), kv index 1.
    wv = Wkv.reshape(E, G, 2, D)[:, :, 1, :].reshape(E, NV)  # (2048, 256)
    bv = bkv.reshape(G, 2, D)[:, 1, :].reshape(NV)  # (256,)

    wv_dev = np.ascontiguousarray(
        wv.reshape(KO, 128, NV).transpose(1, 0, 2)
    ).astype(BF16NP)  # (128, KO, NV): wv_dev[p, ko, n] = Wv[ko*128+p, n]
    bias_dev = np.ascontiguousarray(
        np.broadcast_to(bv[None, :], (128, NV))
    ).astype(np.float32)

    # x tokens: cast bf16, then (core, tt, t, ko, k) -> per-core [tt, k, ko, t]
    xb = x.astype(BF16NP)
    xt = xb.reshape(NCORES, TT, 128, KO, 128)
    xt = np.ascontiguousarray(xt.transpose(0, 1, 4, 3, 2))
    return xt, wv_dev, bias_dev


def kernel(x, Wq, bq, Wkv, bkv):
    global LAST_RESULTS
    from concourse.bass_utils import run_bass_kernel_spmd

    nc = _get_nc()
    xt, wv_dev, bias_dev = _prep_inputs(x, Wkv, bkv)
    in_maps = [
        {"xt": xt[c], "wv": wv_dev, "bias": bias_dev} for c in range(NCORES)
    ]
    res = run_bass_kernel_spmd(nc, in_maps, core_ids=list(range(NCORES)))
    LAST_RESULTS = res
    out = np.concatenate(
        [
            res.results[c]["out"].reshape(TPC, E).astype(np.float32)
            for c in range(NCORES)
        ],
        axis=0,
    )
    return np.ascontiguousarray(out.reshape(B, L, E))
